# revision 15
# baseline (speedup 1.0000x reference)
"""Trainium2 Bass kernel for causal sliding-window self-attention.

Shapes (hardcoded): B=2, T=2048, NH=12, HD=128, HIDDEN=1536, window=1024.

Sharding: 8 cores; core c handles batch b=c//4 and heads [3*(c%4), 3*(c%4)+3).
Each core computes q/k/v projections for its 3 heads (contraction over the
full hidden dim), RoPE + RMS-norm, block-sparse attention (query block i
attends key blocks [i-8, i]), and a partial output projection. The host sums
the 4 partial projections per batch element. No collectives.

Layout strategy per head:
  - project q,k,v into [T, d] (token-major) so RoPE/RMS-norm reductions are
    free-dim reductions; PE-transpose q,k to [d, T] (fp16, 2-wait-slot safe)
  - scores computed transposed: s_t[c, r] = k_tile.T @ q_pair  (so p @ v needs
    no transpose of p);  q pre-scaled by rms_q, k pre-scaled by rms_k/sqrt(HD)
  - softmax denominator: a ones-column appended to v accumulates sum(p) in the
    same PSUM tile as p@v
  - exp computed as exp(s - 3) (softmax shift-invariant) to keep fp16 p finite
"""

import sys
import os

sys.path.insert(0, "/opt/trn_rl_repo")

import numpy as np
from contextlib import ExitStack

import concourse.bass as bass
import concourse.bacc as bacc
import concourse.tile as tile
from concourse import mybir
from concourse.bass_utils import run_bass_kernel_spmd

F32 = mybir.dt.float32
F16 = mybir.dt.float16
AF = mybir.ActivationFunctionType

B, T, NH, HD = 2, 2048, 12, 128
HIDDEN = NH * HD
EPS = 1.1920928955078125e-07
NB = T // 128        # 16 token blocks
KT = HIDDEN // 128   # 12 contraction tiles
WB = 8               # window in blocks (1024/128)
NHC = 3              # heads per core
EXP_SHIFT = -3.0     # exp(s + EXP_SHIFT); cancels in softmax, keeps fp16 finite

_cached_nc = None
_CFG = {"dr8": 1, "o16": 1}


def _window(i):
    return list(range(max(0, i - WB), i + 1))


def _build(stages="ABC", cfg=None):
    cfg = dict(cfg) if cfg else {}
    cfg.setdefault("pipe", 0)
    cfg.setdefault("dr8", _CFG.get("dr8", 1))  # qkv: 3-pass hi/lo fp8 DoubleRow
    cfg.setdefault("newton", 1)  # rsqrt via Pool Newton (no Sqrt act table)
    cfg.setdefault("cpool", 0)   # Pool cannot read PSUM: keep copies on DVE
    cfg.setdefault("o16", _CFG.get("o16", 1))  # fp16 output DMA (host converts)
    if cfg["odma"]:
        cfg["o16"] = 0
    cfg.setdefault("grp", 4)     # key blocks per scores/exp group
    cfg.setdefault("odma", 0)    # out-proj: DMA straight from PSUM (fp32 out)
    nc = bacc.Bacc("TRN2", target_bir_lowering=False, debug=False, num_devices=8)

    dr8 = cfg["dr8"]
    F8 = mybir.dt.float8e4
    if dr8:
        x8h = nc.dram_tensor("x8h", [HIDDEN, T], F8, kind="ExternalInput")
        x8l = nc.dram_tensor("x8l", [HIDDEN, T], F8, kind="ExternalInput")
        wq8h = nc.dram_tensor("wq8h", [HIDDEN, NHC * HD], F8, kind="ExternalInput")
        wq8l = nc.dram_tensor("wq8l", [HIDDEN, NHC * HD], F8, kind="ExternalInput")
        wk8h = nc.dram_tensor("wk8h", [HIDDEN, NHC * HD], F8, kind="ExternalInput")
        wk8l = nc.dram_tensor("wk8l", [HIDDEN, NHC * HD], F8, kind="ExternalInput")
        wv8h = nc.dram_tensor("wv8h", [HIDDEN, NHC * HD], F8, kind="ExternalInput")
        wv8l = nc.dram_tensor("wv8l", [HIDDEN, NHC * HD], F8, kind="ExternalInput")
    else:
        x16 = nc.dram_tensor("x16", [HIDDEN, T], F16, kind="ExternalInput")
        wq16 = nc.dram_tensor("wq16", [HIDDEN, NHC * HD], F16, kind="ExternalInput")
        wk16 = nc.dram_tensor("wk16", [HIDDEN, NHC * HD], F16, kind="ExternalInput")
        wv16 = nc.dram_tensor("wv16", [HIDDEN, NHC * HD], F16, kind="ExternalInput")
    wp16 = nc.dram_tensor("wp16", [NHC * HD, HIDDEN], F16, kind="ExternalInput")
    cos3 = nc.dram_tensor("cos3", [T, NHC * 64], F32, kind="ExternalInput")
    sin3 = nc.dram_tensor("sin3", [T, NHC * 64], F32, kind="ExternalInput")
    mdiag = nc.dram_tensor("mdiag", [128, 128], F16, kind="ExternalInput")
    medge = nc.dram_tensor("medge", [128, 128], F16, kind="ExternalInput")
    ident = nc.dram_tensor("ident", [128, 128], F16, kind="ExternalInput")
    outp = nc.dram_tensor(
        "outp", [T, HIDDEN], F16 if cfg["o16"] else F32, kind="ExternalOutput"
    )

    with tile.TileContext(nc) as tc:
        with ExitStack() as ctx:
            const = ctx.enter_context(tc.tile_pool(name="const", bufs=1))
            persist = ctx.enter_context(tc.tile_pool(name="persist", bufs=1))

            # --- constants / weights -------------------------------------
            if dr8:
                wqth = const.tile([128, KT, NHC * HD], F8)
                wqtl = const.tile([128, KT, NHC * HD], F8)
                wkth = const.tile([128, KT, NHC * HD], F8)
                wktl = const.tile([128, KT, NHC * HD], F8)
                wvth = const.tile([128, KT, NHC * HD], F8)
                wvtl = const.tile([128, KT, NHC * HD], F8)
                for dst_, src_ in (
                    (wqth, wq8h), (wqtl, wq8l), (wkth, wk8h),
                    (wktl, wk8l), (wvth, wv8h), (wvtl, wv8l),
                ):
                    nc.sync.dma_start(
                        dst_[:], src_.ap().rearrange("(k p) n -> p k n", p=128)
                    )
            else:
                wqt = const.tile([128, KT, NHC * HD], F16)
                wkt = const.tile([128, KT, NHC * HD], F16)
                wvt = const.tile([128, KT, NHC * HD], F16)
                nc.sync.dma_start(
                    wqt[:], wq16.ap().rearrange("(k p) n -> p k n", p=128)
                )
                nc.sync.dma_start(
                    wkt[:], wk16.ap().rearrange("(k p) n -> p k n", p=128)
                )
                nc.sync.dma_start(
                    wvt[:], wv16.ap().rearrange("(k p) n -> p k n", p=128)
                )
            wpt = const.tile([128, NHC, HIDDEN], F16)
            nc.sync.dma_start(wpt[:], wp16.ap().rearrange("(k p) n -> p k n", p=128))
            cost = const.tile([128, NB, NHC, 64], F32)
            sint = const.tile([128, NB, NHC, 64], F32)
            nc.sync.dma_start(
                cost[:], cos3.ap().rearrange("(m p) (h c) -> p m h c", p=128, h=NHC)
            )
            nc.sync.dma_start(
                sint[:], sin3.ap().rearrange("(m p) (h c) -> p m h c", p=128, h=NHC)
            )
            mdg = const.tile([128, 128], F16)
            medg = const.tile([128, 128], F16)
            idt = const.tile([128, 128], F16)
            nc.sync.dma_start(mdg[:], mdiag.ap())
            nc.sync.dma_start(medg[:], medge.ap())
            nc.sync.dma_start(idt[:], ident.ap())
            biast = const.tile([128, 4], F32)
            nc.vector.memset(biast[:, 0:1], EPS)
            nc.vector.memset(biast[:, 1:2], HD * EPS)
            nc.vector.memset(biast[:, 2:3], EXP_SHIFT)
            b_eps = biast[:, 0:1]
            b_epsk = biast[:, 1:2]
            b_shift = biast[:, 2:3]

            # --- persistent per-head tensors -----------------------------
            qt = persist.tile([128, NHC, T], F16)   # q^T  [d, t] per head
            kt = persist.tile([128, NHC, T], F16)   # k^T  [d, t] per head
            yt = persist.tile([128, NHC, T], F16)   # y^T  [d, t] per head
            vext = persist.tile([128, NHC, NB, 132], F16)  # v tiles + ones col
            nc.gpsimd.memset(vext[:], 1.0)  # col 128 stays 1.0; 0:128 overwritten

            if dr8:
                x8hr = x8h.ap().rearrange("(k p) t -> p k t", p=128)
                x8lr = x8l.ap().rearrange("(k p) t -> p k t", p=128)
            else:
                x16r = x16.ap().rearrange("(k p) t -> p k t", p=128)
            # constants for folding the 64x weight pre-scale (dr8) and the
            # 1/sqrt(HD) score scale (k side) into the rq write
            WS = 64.0 if dr8 else 1.0
            C_Q = 1.0 / WS
            C_K = 1.0 / (WS * np.sqrt(128.0))
            SQ_SCALE = 1.0 / (WS * np.sqrt(128.0))  # (x*s)^2 summed -> mean(q^2)

            # pools for the fused loop
            xp = ctx.enter_context(tc.tile_pool(name="xp", bufs=cfg.get("xp", 4)))
            rp = ctx.enter_context(tc.tile_pool(name="rp", bufs=cfg.get("rp", 3)))
            pp = ctx.enter_context(tc.tile_pool(name="pp", bufs=cfg.get("pp", 12)))
            yp = ctx.enter_context(tc.tile_pool(name="yp", bufs=cfg.get("yp", 3)))
            op_sb = ctx.enter_context(tc.tile_pool(name="opsb", bufs=cfg.get("osb", 3)))
            psA = ctx.enter_context(
                tc.tile_pool(name="psA", bufs=cfg.get("psA", 1), space="PSUM")
            )
            tpps = ctx.enter_context(
                tc.tile_pool(name="tpps", bufs=cfg.get("tpps", 1), space="PSUM")
            )
            spsum = ctx.enter_context(
                tc.tile_pool(name="spsum", bufs=cfg.get("sps", 2), space="PSUM")
            )
            opsum = ctx.enter_context(
                tc.tile_pool(name="opsum", bufs=cfg.get("ops", 1), space="PSUM")
            )
            cps = ctx.enter_context(
                tc.tile_pool(name="cps", bufs=cfg.get("cps", 1), space="PSUM")
            )

            def stage_a(m):
                psq = psA.tile([128, NHC, HD], F32, tag="psq")
                psk = psA.tile([128, NHC, HD], F32, tag="psk")
                psv = psA.tile([128, NHC, HD], F32, tag="psv")
                if dr8:
                    xmh = xp.tile([128, KT, 128], F8, tag="xmh")
                    xml = xp.tile([128, KT, 128], F8, tag="xml")
                    nc.sync.dma_start(xmh[:], x8hr[:, :, m * 128 : (m + 1) * 128])
                    nc.sync.dma_start(xml[:], x8lr[:, :, m * 128 : (m + 1) * 128])
                    DR = mybir.MatmulPerfMode.DoubleRow
                    # 3-pass hi/lo: xh*wh + xl*wh + xh*wl, 6 chunks of 256
                    passes = [(xmh, 0), (xml, 0), (xmh, 1)]
                    nch = KT // 2
                    last = 3 * nch - 1
                    for ps_, wth_, wtl_ in (
                        (psq, wqth, wqtl), (psk, wkth, wktl), (psv, wvth, wvtl),
                    ):
                        idx = 0
                        for xt_, wl_ in passes:
                            wt_ = (wth_, wtl_)[wl_]
                            for kk in range(nch):
                                nc.tensor.matmul(
                                    ps_[:],
                                    xt_[:, 2 * kk : 2 * kk + 2, :],
                                    wt_[:, 2 * kk : 2 * kk + 2, :],
                                    start=(idx == 0), stop=(idx == last),
                                    perf_mode=DR,
                                )
                                idx += 1
                else:
                    xm = xp.tile([128, KT, 128], F16, tag="xm")
                    nc.sync.dma_start(xm[:], x16r[:, :, m * 128 : (m + 1) * 128])
                    for kk in range(KT):
                        nc.tensor.matmul(
                            psq[:], xm[:, kk, :], wqt[:, kk, :],
                            start=(kk == 0), stop=(kk == KT - 1),
                        )
                        nc.tensor.matmul(
                            psk[:], xm[:, kk, :], wkt[:, kk, :],
                            start=(kk == 0), stop=(kk == KT - 1),
                        )
                        nc.tensor.matmul(
                            psv[:], xm[:, kk, :], wvt[:, kk, :],
                            start=(kk == 0), stop=(kk == KT - 1),
                        )
                cosm = cost[:, m]  # [128, 3, 64]
                sinm = sint[:, m]
                MUL = mybir.AluOpType.mult
                ADD = mybir.AluOpType.add
                ny = None
                if cfg["newton"]:
                    # mean-of-squares from the (pre-rope) projections via Act
                    # Square+accum (rope preserves per-token norms; Square
                    # shares the act table with Exp so no table reloads),
                    # then rsqrt via Newton on the (idle) Pool engine:
                    # y1 = 1.5 - 0.5 v;  y <- y*(1.5 - 0.5 v y^2)  (4 iters)
                    sqs = rp.tile([128, 8], F32, tag="sqs")
                    scr = rp.tile([128, NHC, HD], F32, tag="scr")
                    for si, src in ((0, psq), (4, psk)):
                        for hh in range(NHC):
                            nc.scalar.activation(
                                scr[:, hh, :], src[:, hh, :], AF.Square,
                                scale=SQ_SCALE,
                                accum_out=sqs[:, si + hh : si + hh + 1],
                            )
                    ny = rp.tile([128, 8], F32, tag="ny")
                    nt = rp.tile([128, 8], F32, tag="nt")
                    nz = rp.tile([128, 8], F32, tag="nz")
                    # first Newton step fused for seed y0=0.8 (v in ~[0.7,2.4]):
                    # y1 = y0*(1.5 - 0.5*v*y0^2) = 1.2 - 0.256*v
                    nc.gpsimd.tensor_scalar(ny[:], sqs[:], -0.256, 1.2, MUL, ADD)
                    for _ in range(4):
                        nc.gpsimd.tensor_mul(nt[:], ny[:], ny[:])
                        nc.gpsimd.tensor_mul(nt[:], nt[:], sqs[:])
                        nc.gpsimd.tensor_scalar(nz[:], nt[:], -0.5, 1.5, MUL, ADD)
                        nc.gpsimd.tensor_mul(ny[:], ny[:], nz[:])
                for src, si, cc, b_rms, scale_rms, dst in (
                    (psq, 0, C_Q, b_eps, 1.0 / HD, qt),
                    (psk, 4, C_K, b_epsk, 1.0, kt),
                ):
                    t1 = rp.tile([128, NHC, 64], F32, tag="t1")
                    t2 = rp.tile([128, NHC, 64], F32, tag="t2")
                    t3 = rp.tile([128, NHC, 64], F32, tag="t3")
                    t4 = rp.tile([128, NHC, 64], F32, tag="t4")
                    u = rp.tile([128, NHC, HD], F32, tag="u")
                    nc.vector.tensor_mul(t1[:], src[:, :, 0:64], cosm)
                    nc.vector.tensor_mul(t2[:], src[:, :, 64:128], sinm)
                    nc.vector.tensor_add(u[:, :, 0:64], t1[:], t2[:])
                    nc.vector.tensor_mul(t3[:], src[:, :, 64:128], cosm)
                    nc.vector.tensor_mul(t4[:], src[:, :, 0:64], sinm)
                    nc.vector.tensor_sub(u[:, :, 64:128], t3[:], t4[:])
                    if not cfg["newton"]:
                        sqs = rp.tile([128, 4], F32, tag="sqs")
                        scq = rp.tile([128, NHC, HD], F32, tag="scq")
                        nc.vector.tensor_mul(scq[:], u[:], u[:])
                        nc.vector.tensor_reduce(
                            sqs[:, 0:3], scq[:],
                            mybir.AxisListType.X, mybir.AluOpType.add,
                        )
                        root = rp.tile([128, 4], F32, tag="root")
                        nc.scalar.activation(
                            root[:, 0:3], sqs[:, 0:3], AF.Sqrt,
                            bias=b_rms, scale=scale_rms,
                        )
                        rs = rp.tile([128, 4], F32, tag="rs")
                        nc.vector.reciprocal(rs[:, 0:3], root[:, 0:3])
                    rq = rp.tile([128, NHC, HD], F16, tag="rq")
                    for hh in range(NHC):
                        if cfg["newton"]:
                            nc.gpsimd.tensor_scalar(
                                rq[:, hh, :], u[:, hh, :],
                                ny[:, si + hh : si + hh + 1], cc, MUL, MUL,
                            )
                        else:
                            nc.vector.tensor_scalar_mul(
                                rq[:, hh, :], u[:, hh, :], rs[:, hh : hh + 1]
                            )
                    for hh in range(NHC):
                        if cfg.get("dmat", 0):
                            nc.sync.dma_start(
                                dst[:, hh, m * 128 : (m + 1) * 128],
                                rq[:, hh, :], transpose=True,
                            )
                        else:
                            tp = tpps.tile([128, 128], F16, tag="tp")
                            nc.tensor.transpose(tp[:], rq[:, hh, :], idt[:])
                            if cfg["cpool"]:
                                nc.gpsimd.tensor_copy(
                                    dst[:, hh, m * 128 : (m + 1) * 128], tp[:]
                                )
                            else:
                                nc.vector.tensor_copy(
                                    dst[:, hh, m * 128 : (m + 1) * 128], tp[:]
                                )
                for hh in range(NHC):
                    if cfg.get("vact", 1):
                        nc.scalar.copy(vext[:, hh, m, 0:128], psv[:, hh, :])
                    else:
                        nc.vector.tensor_copy(vext[:, hh, m, 0:128], psv[:, hh, :])

            def attention_pair(pr):
                i0, i1 = 2 * pr, 2 * pr + 1
                js = list(range(max(0, i0 - WB), i1 + 1))
                for hh in range(NHC):
                    ptloc = {}
                    for g0 in range(0, len(js), 2):
                        grp = js[g0 : g0 + 2]
                        w = len(grp) * 256
                        sps = spsum.tile([128, 512], F32, tag="sps")
                        for gi, j in enumerate(grp):
                            nc.tensor.matmul(
                                sps[:, gi * 256 : (gi + 1) * 256],
                                kt[:, hh, j * 128 : (j + 1) * 128],
                                qt[:, hh, i0 * 128 : (i0 + 2) * 128],
                                start=True, stop=True,
                            )
                        ptile = pp.tile([128, 512], F16, tag="pt")
                        nc.scalar.activation(
                            ptile[:, 0:w], sps[:, 0:w], AF.Exp,
                            bias=b_shift, scale=1.0,
                        )
                        for gi, j in enumerate(grp):
                            ptloc[j] = (ptile, gi * 256)

                    def mask_mult(j, half, mask):
                        t, off = ptloc[j]
                        o = off + half * 128
                        nc.gpsimd.tensor_mul(
                            t[:, o : o + 128], t[:, o : o + 128], mask[:]
                        )

                    mask_mult(i0, 0, mdg)
                    mask_mult(i1, 1, mdg)
                    if i0 >= WB:
                        mask_mult(i0 - WB, 0, medg)
                    if i1 >= WB:
                        mask_mult(i1 - WB, 1, medg)

                    for half, i in enumerate((i0, i1)):
                        jsi = _window(i)
                        ops = opsum.tile([128, 132], F32, tag="ops")
                        for idx, j in enumerate(jsi):
                            t, off = ptloc[j]
                            o = off + half * 128
                            nc.tensor.matmul(
                                ops[:, 0:129],
                                t[:, o : o + 128],
                                vext[:, hh, j, 0:129],
                                start=(idx == 0), stop=(idx == len(jsi) - 1),
                            )
                        rden = yp.tile([128, 1], F32, tag="rden")
                        nc.vector.reciprocal(rden[:], ops[:, 128:129])
                        ysb = yp.tile([128, 128], F16, tag="ysb")
                        nc.vector.tensor_scalar_mul(ysb[:], ops[:, 0:128], rden[:])
                        if cfg.get("dmat", 0):
                            nc.sync.dma_start(
                                yt[:, hh, i * 128 : (i + 1) * 128],
                                ysb[:], transpose=True,
                            )
                        else:
                            ytp = tpps.tile([128, 128], F16, tag="tp")
                            nc.tensor.transpose(ytp[:], ysb[:], idt[:])
                            if cfg["cpool"]:
                                nc.gpsimd.tensor_copy(
                                    yt[:, hh, i * 128 : (i + 1) * 128], ytp[:]
                                )
                            else:
                                nc.vector.tensor_copy(
                                    yt[:, hh, i * 128 : (i + 1) * 128], ytp[:]
                                )

            def out_proj(m):
                if not cfg["odma"]:
                    osb = op_sb.tile(
                        [128, HIDDEN], F16 if cfg["o16"] else F32, tag="osb"
                    )
                for n in range(3):
                    po = cps.tile([128, 512], F32, tag="po")
                    for hh in range(NHC):
                        nc.tensor.matmul(
                            po[:],
                            yt[:, hh, m * 128 : (m + 1) * 128],
                            wpt[:, hh, n * 512 : (n + 1) * 512],
                            start=(hh == 0), stop=(hh == NHC - 1),
                        )
                    if cfg["odma"]:
                        nc.sync.dma_start(
                            outp.ap()[
                                m * 128 : (m + 1) * 128, n * 512 : (n + 1) * 512
                            ],
                            po[:],
                        )
                    elif n % 2 == 0:
                        nc.vector.tensor_copy(osb[:, n * 512 : (n + 1) * 512], po[:])
                    else:
                        nc.scalar.copy(osb[:, n * 512 : (n + 1) * 512], po[:])
                if not cfg["odma"]:
                    nc.sync.dma_start(outp.ap()[m * 128 : (m + 1) * 128, :], osb[:])

            # fused pair-major schedule (optionally software-pipelined by one
            # pair: attention/out-proj for pair pr-1 run between A-pairs)
            nreps = cfg.get("reps", 0)
            repctx = tc.For_i(0, nreps, 1) if nreps else None
            if repctx is not None:
                repctx.__enter__()
            if cfg.get("pipe", 0):
                stage_a(0)
                stage_a(1)
                for pr in range(1, NB // 2):
                    stage_a(2 * pr)
                    stage_a(2 * pr + 1)
                    attention_pair(pr - 1)
                    out_proj(2 * pr - 2)
                    out_proj(2 * pr - 1)
                attention_pair(NB // 2 - 1)
                out_proj(NB - 2)
                out_proj(NB - 1)
            else:
                for pr in range(NB // 2):
                    stage_a(2 * pr)
                    stage_a(2 * pr + 1)
                    attention_pair(pr)
                    out_proj(2 * pr)
                    out_proj(2 * pr + 1)
            if repctx is not None:
                repctx.__exit__(None, None, None)

    nc.compile()
    return nc


def _get_nc():
    global _cached_nc
    if _cached_nc is None:
        _cached_nc = _build()
    return _cached_nc


def _rope_tables():
    d_half = HD // 2
    inv = 1.0 / (10000.0 ** (np.arange(d_half, dtype=np.float64) / d_half))
    t = np.arange(T, dtype=np.float64)
    f = t[:, None] * inv[None, :]
    return np.cos(f), np.sin(f)


def kernel(x, cos, sin, Wq, Wk, Wv, Wp, window, _trace=False, _result_holder=None):
    x = np.asarray(x, dtype=np.float32)
    cos = np.asarray(cos, dtype=np.float32)
    sin = np.asarray(sin, dtype=np.float32)
    Wq = np.asarray(Wq, dtype=np.float32)
    Wk = np.asarray(Wk, dtype=np.float32)
    Wv = np.asarray(Wv, dtype=np.float32)
    Wp = np.asarray(Wp, dtype=np.float32)
    assert int(window) == 1024, f"kernel hardcodes window=1024, got {window}"

    cosn = np.ascontiguousarray(cos[0, :, 0, :])  # [T, 64]
    sinn = np.ascontiguousarray(sin[0, :, 0, :])
    cos3 = np.tile(cosn, (1, NHC)).astype(np.float32)  # [T, 192]
    sin3 = np.tile(sinn, (1, NHC)).astype(np.float32)

    c = np.arange(128)[:, None]
    r = np.arange(128)[None, :]
    mdiag = (c <= r).astype(np.float16)
    medge = (r <= c).astype(np.float16)
    ident = np.eye(128, dtype=np.float16)

    dr8 = _CFG.get("dr8", 1)
    if dr8:
        import ml_dtypes

        F8NP = ml_dtypes.float8_e4m3fn

        def hilo(a):
            a = np.ascontiguousarray(a).astype(np.float32)
            hi = a.astype(F8NP)
            lo = (a - hi.astype(np.float32)).astype(F8NP)
            return hi, lo

        xsp = {b: hilo(x[b].T) for b in range(B)}
        wsp = {}
        for g in range(4):
            S = slice(g * NHC * HD, (g + 1) * NHC * HD)
            wsp[g] = (
                hilo(Wq[S, :].T * 64.0),
                hilo(Wk[S, :].T * 64.0),
                hilo(Wv[S, :].T * 64.0),
            )

    in_maps = []
    for core in range(8):
        b = core // 4
        g = core % 4
        S = slice(g * NHC * HD, (g + 1) * NHC * HD)
        m = {
            "cos3": cos3,
            "sin3": sin3,
            "mdiag": mdiag,
            "medge": medge,
            "ident": ident,
        }
        if dr8:
            (xh, xl), ((wqh, wql), (wkh, wkl), (wvh, wvl)) = xsp[b], wsp[g]
            m.update(
                x8h=xh, x8l=xl, wq8h=wqh, wq8l=wql,
                wk8h=wkh, wk8l=wkl, wv8h=wvh, wv8l=wvl,
                wp16=np.ascontiguousarray(Wp[:, S].T / 64.0).astype(np.float16),
            )
        else:
            m.update(
                x16=np.ascontiguousarray(x[b].T).astype(np.float16),
                wq16=np.ascontiguousarray(Wq[S, :].T).astype(np.float16),
                wk16=np.ascontiguousarray(Wk[S, :].T).astype(np.float16),
                wv16=np.ascontiguousarray(Wv[S, :].T).astype(np.float16),
                wp16=np.ascontiguousarray(Wp[:, S].T).astype(np.float16),
            )
        in_maps.append(m)

    nc = _get_nc()
    res = run_bass_kernel_spmd(nc, in_maps, list(range(8)), trace=_trace)
    if _result_holder is not None:
        _result_holder.append(res)

    out = np.zeros((B, T, HIDDEN), dtype=np.float32)
    for core in range(8):
        out[core // 4] += np.asarray(res.results[core]["outp"], dtype=np.float32)
    return out

